# revision 17
# baseline (speedup 1.0000x reference)
"""GAT message-passing kernel for Trainium2 (8 NeuronCores, Bass/Tile).

Strategy (dst-sharded per the spec sharding hint):
  - Host (indexing/layout only): sort edges by dst, shard dst ranges across 8
    cores, group each core's edges into 128-dst-node blocks, split each block's
    edges by src table half (int16 gather indices), pad to fixed chunk counts.
    h is shipped transposed in fp16 (layout prep for the device matmuls).
  - Device phase 1 (per core, full N): hW = h @ W plus er/el attention halves
    via PE matmuls (lhsT = host-provided h^T tiles); writes fp16 DRAM tables
    aug = [hW | er | pad] (512B rows, gathered by src) and el[N, 4].
  - Device phase 2 (per core, its dst blocks): dma_gather of aug rows by src
    (two table halves), el expanded node->edge on chip via a K=1 broadcast
    matmul + one-hot S^T matmul; e = lrelu(er+el+b); w = exp(e); segment
    softmax + weighted scatter-sum via one-hot matmul per 128-edge chunk into
    PSUM [128 dst x (128 msg | 4 denom)]; out = num/denom + bias.
"""

import math
import os
import sys

import numpy as np

for _p in ("/opt/trn_rl_repo",):
    if _p not in sys.path:
        sys.path.insert(0, _p)

import concourse.bacc as bacc
import concourse.mybir as mybir
import concourse.tile as tile
from concourse.bass_utils import run_bass_kernel_spmd

F32 = mybir.dt.float32
F16 = mybir.dt.float16
I16 = mybir.dt.int16
I32 = mybir.dt.int32
AF = mybir.ActivationFunctionType
AL = mybir.AluOpType

# Problem dims (hardcoded per spec nn_GAT_1090921693566)
N, E, D, H, F = 50000, 800000, 256, 4, 32
HF = H * F  # 128
NEG_SLOPE = 0.2
NCORES = 8
P = 128
ROWE = 256  # fp16 elements per aug row (512B): [hW 128 | er 4 | pad 124]


def build_program(n_pad, halfr, nblk, cpba, cpbb, npc, g4=8):
    """Single-core SPMD program.

    n_pad: padded node count (= 2*halfr), table rows
    halfr: rows per table half (< 32768 for int16 gather indices)
    nblk:  dst blocks per core (128 nodes each)
    cpba/cpbb: 128-edge chunks per block gathered from half A / half B
    npc:   nodes owned per core (el shard offset = core_id * npc)
    """
    cpb = cpba + cpbb
    ntile = n_pad // P
    ngrp = math.ceil(ntile / g4)
    ecols = (cpba + cpbb) * 8 + cpb  # eidx free width (wrapped idxs + drel)

    nc = bacc.Bacc("TRN2")

    ht_d = nc.declare_dram_parameter("hT16", [D, n_pad], F16, isOutput=False)
    w_d = nc.declare_dram_parameter("W", [D, HF], F32, isOutput=False)
    arb_d = nc.declare_dram_parameter("attn_r_bcast", [P, HF], F16, isOutput=False)
    alb_d = nc.declare_dram_parameter("attn_l_bcast", [P, HF], F16, isOutput=False)
    bsum_d = nc.declare_dram_parameter("bsum", [P, 1], F32, isOutput=False)
    biasb_d = nc.declare_dram_parameter("bias_bcast", [P, HF], F32, isOutput=False)
    pid_d = nc.declare_dram_parameter("pidrow", [1, 1], I32, isOutput=False)
    eidx_d = nc.declare_dram_parameter("eidx", [nblk, P, ecols], I16, isOutput=False)
    drow_d = nc.declare_dram_parameter("drow", [nblk, cpb * P], F16, isOutput=False)
    out_d = nc.declare_dram_parameter("out", [nblk * P, HF], F32, isOutput=True)

    with tile.TileContext(nc) as tc:
        with (
            tc.tile_pool(name="const", bufs=1) as cp,
            tc.tile_pool(name="dramtab", bufs=1, space="DRAM") as dp,
        ):
            aug_t = dp.tile([n_pad, ROWE], F16)  # [hW | er | pad]
            el_t = dp.tile([n_pad, 4], F16)

            iota_i = cp.tile([P, P], I32)
            nc.gpsimd.iota(iota_i[:], pattern=[[1, P]], base=0, channel_multiplier=0)
            iota_row = cp.tile([P, P], F16)
            nc.vector.tensor_copy(iota_row[:], iota_i[:])
            iotap_i = cp.tile([P, 1], I32)
            nc.gpsimd.iota(iotap_i[:], pattern=[[0, 1]], base=0, channel_multiplier=1)
            iota_part = cp.tile([P, 1], F32)
            nc.vector.tensor_copy(iota_part[:], iotap_i[:])
            ones_col = cp.tile([1, P], F16)
            nc.gpsimd.memset(ones_col[:], 1.0)

            arb = cp.tile([P, HF], F16)
            nc.sync.dma_start(out=arb[:], in_=arb_d[:])
            alb = cp.tile([P, HF], F16)
            nc.sync.dma_start(out=alb[:], in_=alb_d[:])
            bsum = cp.tile([P, 1], F32)
            nc.sync.dma_start(out=bsum[:], in_=bsum_d[:])
            biasb = cp.tile([P, HF], F32)
            nc.sync.dma_start(out=biasb[:], in_=biasb_d[:])

            # Waug[k] = [W rows k*128.. | wr | wl] fp16; wr = W@attn_r per head
            waug = cp.tile([P, 2, HF + 8], F16)
            tmp_r = cp.tile([P, HF], F16)
            tmp_l = cp.tile([P, HF], F16)
            for k in range(2):
                # cast-load W fp32 -> fp16 (SWDGE cast)
                nc.gpsimd.dma_start(
                    out=waug[:, k, 0:HF], in_=w_d[k * P : (k + 1) * P, :]
                )
                nc.vector.tensor_mul(tmp_r[:], waug[:, k, 0:HF], arb[:])
                nc.vector.tensor_mul(tmp_l[:], waug[:, k, 0:HF], alb[:])
                with nc.allow_low_precision(
                    reason="wr/wl: 32-elem fp16 sums of ~unit values"
                ):
                    nc.vector.reduce_sum(
                        waug[:, k, HF : HF + 4],
                        tmp_r.rearrange("p (h f) -> p h f", f=F),
                        axis=mybir.AxisListType.X,
                    )
                    nc.vector.reduce_sum(
                        waug[:, k, HF + 4 : HF + 8],
                        tmp_l.rearrange("p (h f) -> p h f", f=F),
                        axis=mybir.AxisListType.X,
                    )

            # ---------------- Phase 1: build aug/el tables ----------------
            with (
                tc.tile_pool(name="p1", bufs=3) as p1,
                tc.tile_pool(name="ps1", bufs=4, space="PSUM") as ps1,
            ):
                for g in range(ngrp):
                    t0 = g * g4
                    gtiles = min(g4, ntile - t0)
                    col0 = t0 * P
                    ncols = gtiles * P
                    # hT tile: [d, nodes] fp16 native (host-transposed)
                    ht_sb = p1.tile([P, 2, g4 * P], F16, tag="ht_sb")
                    nc.sync.dma_start(
                        out=ht_sb[:, :, 0:ncols],
                        in_=ht_d[:, col0 : col0 + ncols].rearrange(
                            "(k p) n -> p k n", p=P
                        ),
                    )
                    aug_sb = p1.tile([P, g4, HF + 8], F16, tag="aug_sb")
                    for tl in range(gtiles):
                        ps_aug = ps1.tile([P, HF + 8], F32, tag="psaug")
                        for k in range(2):
                            nc.tensor.matmul(
                                ps_aug[:],
                                lhsT=ht_sb[:, k, tl * P : (tl + 1) * P],
                                rhs=waug[:, k, :],
                                start=(k == 0),
                                stop=(k == 1),
                            )
                        nc.vector.tensor_copy(aug_sb[:, tl, :], ps_aug[:])
                    nc.scalar.dma_start(
                        out=aug_t[col0 : col0 + ncols, 0 : HF + 4].rearrange(
                            "(t p) c -> p t c", p=P
                        ),
                        in_=aug_sb[:, 0:gtiles, 0 : HF + 4],
                    )
                    nc.scalar.dma_start(
                        out=el_t[col0 : col0 + ncols, :].rearrange(
                            "(t p) c -> p t c", p=P
                        ),
                        in_=aug_sb[:, 0:gtiles, HF + 4 : HF + 8],
                    )

            # el for own dst shard -> SBUF resident [128, nblk, 4]
            import concourse.bass as bass

            el_own = cp.tile([P, nblk, 4], F16)
            with nc.gpsimd.register("pidreg") as pidreg:
                nc.gpsimd.reg_load(pidreg, pid_d[0:1, 0:1])
                with nc.gpsimd.register("rowoff") as rowoff:
                    nc.gpsimd.reg_alu(rowoff, pidreg, npc, AL.mult)
                    off = nc.gpsimd.snap(rowoff)
                    nc.gpsimd.dma_start(
                        out=el_own[:],
                        in_=el_t[bass.ds(off, nblk * P), :].rearrange(
                            "(b p) h -> p b h", p=P
                        ),
                    )

            # ---------------- Phase 2: edge blocks ----------------
            with (
                tc.tile_pool(name="p2", bufs=3) as p2,
                tc.tile_pool(name="psb", bufs=1, space="PSUM") as psb,
                tc.tile_pool(name="pse", bufs=1, space="PSUM") as pse,
                tc.tile_pool(name="psm", bufs=2, space="PSUM") as psm,
            ):
                for b in range(nblk):
                    eidx = p2.tile([P, ecols], I16, tag="eidx")
                    nc.sync.dma_start(out=eidx[:], in_=eidx_d[b])
                    drow = p2.tile([1, cpb * P], F16, tag="drow")
                    nc.sync.dma_start(out=drow[:], in_=drow_d[b : b + 1, :])
                    g_t = p2.tile([P, cpb, ROWE], F16, tag="g_t")
                    nc.gpsimd.dma_gather(
                        out_ap=g_t[:, 0:cpba, :],
                        in_ap=aug_t[0:halfr, :],
                        idxs_ap=eidx[:, 0 : cpba * 8],
                        num_idxs=cpba * P,
                        num_idxs_reg=cpba * P,
                        elem_size=ROWE,
                        single_packet=False,
                    )
                    nc.gpsimd.dma_gather(
                        out_ap=g_t[:, cpba:cpb, :],
                        in_ap=aug_t[halfr : 2 * halfr, :],
                        idxs_ap=eidx[:, cpba * 8 : (cpba + cpbb) * 8],
                        num_idxs=cpbb * P,
                        num_idxs_reg=cpbb * P,
                        elem_size=ROWE,
                        single_packet=False,
                    )
                    # S^T via K=1 broadcast matmul of drel row + compare
                    ps_bc = psb.tile([P, cpb * P], F32, tag="ps_bc")
                    for j in range(math.ceil(cpb * P / 512)):
                        lo, hi = j * 512, min((j + 1) * 512, cpb * P)
                        nc.tensor.matmul(
                            ps_bc[:, lo:hi],
                            lhsT=ones_col[:],
                            rhs=drow[:, lo:hi],
                            start=True,
                            stop=True,
                        )
                    s_T = p2.tile([P, cpb, P], F16, tag="s_T")
                    nc.vector.tensor_tensor(
                        out=s_T[:],
                        in0=ps_bc.rearrange("p (c e) -> p c e", e=P),
                        in1=iota_part.to_broadcast([P, cpb, P]),
                        op=AL.is_equal,
                    )
                    # S from drel columns
                    drelf = p2.tile([P, cpb], F16, tag="drelf")
                    nc.vector.tensor_copy(
                        drelf[:], eidx[:, (cpba + cpbb) * 8 : (cpba + cpbb) * 8 + cpb]
                    )
                    s_t = p2.tile([P, cpb, P], F16, tag="s_t")
                    nc.vector.tensor_tensor(
                        out=s_t[:],
                        in0=drelf.to_broadcast([P, cpb, P]),
                        in1=iota_row.rearrange("p (o f) -> p o f", o=1).to_broadcast(
                            [P, cpb, P]
                        ),
                        op=AL.is_equal,
                    )
                    # el per edge: el_edge[e, (c,h)] = sum_n S^T[n,e] el_own[n,b,h]
                    ps_el = pse.tile([P, cpb * 4], F32, tag="ps_el")
                    for c in range(cpb):
                        nc.tensor.matmul(
                            ps_el[:, c * 4 : (c + 1) * 4],
                            lhsT=s_T[:, c, :],
                            rhs=el_own[:, b, :],
                            start=True,
                            stop=True,
                        )
                    # e = er + el; w = exp(lrelu(e + bsum))
                    e32 = p2.tile([P, cpb, 4], F32, tag="e32")
                    nc.vector.tensor_tensor(
                        out=e32[:],
                        in0=ps_el.rearrange("p (c h) -> p c h", h=4),
                        in1=g_t[:, :, HF : HF + 4],
                        op=AL.add,
                    )
                    t2 = p2.tile([P, cpb, 4], F32, tag="t2")
                    nc.vector.tensor_scalar_add(t2[:], e32[:], bsum[:, 0:1])
                    t1 = p2.tile([P, cpb, 4], F32, tag="t1")
                    nc.vector.tensor_scalar(
                        t1[:], e32[:], bsum[:, 0:1], NEG_SLOPE, op0=AL.add, op1=AL.mult
                    )
                    lre = p2.tile([P, cpb, 4], F32, tag="lre")
                    nc.vector.tensor_tensor(out=lre[:], in0=t1[:], in1=t2[:], op=AL.max)
                    r_t = p2.tile([P, cpb, HF + 4], F16, tag="r_t")
                    nc.scalar.activation(r_t[:, :, HF : HF + 4], lre[:], AF.Exp)
                    nc.vector.tensor_tensor(
                        out=r_t[:, :, 0:HF].rearrange("p c (h f) -> p c h f", f=F),
                        in0=g_t[:, :, 0:HF].rearrange("p c (h f) -> p c h f", f=F),
                        in1=r_t[:, :, HF : HF + 4].to_broadcast([P, cpb, 4, F]),
                        op=AL.mult,
                    )
                    ps_m = psm.tile([P, HF + 4], F32, tag="ps_m")
                    for c in range(cpb):
                        nc.tensor.matmul(
                            ps_m[:],
                            lhsT=s_t[:, c, :],
                            rhs=r_t[:, c, :],
                            start=(c == 0),
                            stop=(c == cpb - 1),
                        )
                    den = p2.tile([P, 4], F32, tag="den")
                    nc.vector.tensor_scalar_max(den[:], ps_m[:, HF : HF + 4], 1e-30)
                    rec = p2.tile([P, 4], F32, tag="rec")
                    nc.vector.reciprocal(rec[:], den[:])
                    fin = p2.tile([P, HF], F32, tag="fin")
                    nc.vector.tensor_tensor(
                        out=fin.rearrange("p (h f) -> p h f", f=F),
                        in0=ps_m[:, 0:HF].rearrange("p (h f) -> p h f", f=F),
                        in1=rec.to_broadcast([P, 4, F]),
                        op=AL.mult,
                    )
                    nc.vector.tensor_add(fin[:], fin[:], biasb[:])
                    nc.scalar.dma_start(out=out_d[b * P : (b + 1) * P, :], in_=fin[:])
    return nc


def _wrap16(flat):
    """int16 gather-index list -> [128, len/16] wrapped layout
    (idx[p, s] = flat[s*16 + p%16], replicated across the 8 groups of 16)."""
    n = flat.shape[0]
    assert n % 16 == 0
    w = np.empty((16, n // 16), np.int16)
    for pp in range(16):
        w[pp] = flat[pp::16]
    return np.tile(w, (8, 1))


def preprocess_edges(src, dst, n_nodes, ncores, halfr):
    """Sort edges by dst, shard by dst range, split each 128-dst-node block's
    edges by src half; build per-core eidx ([wrapA|wrapB|drel] int16) and
    drow (fp16 drel in flat chunk order)."""
    npc = n_nodes // ncores
    nblk = math.ceil(npc / P)
    perm = np.argsort(dst, kind="stable")
    ds = dst[perm]
    ss = src[perm]
    core_lo = np.searchsorted(ds, np.arange(ncores + 1) * npc)

    blocks = []  # (core, blk) -> (srcA, drelA, srcB, drelB)
    cpba = cpbb = 1
    for c in range(ncores):
        lo, hi = core_lo[c], core_lo[c + 1]
        rel = ds[lo:hi] - c * npc
        s = ss[lo:hi]
        blk_lo = np.searchsorted(rel, np.arange(nblk + 1) * P)
        per_b = []
        for b in range(nblk):
            l, r = blk_lo[b], blk_lo[b + 1]
            sb = s[l:r]
            rb = rel[l:r] - b * P
            isa = sb < halfr
            sA, dA = sb[isa], rb[isa]
            sB, dB = sb[~isa] - halfr, rb[~isa]
            cpba = max(cpba, (len(sA) + P - 1) // P)
            cpbb = max(cpbb, (len(sB) + P - 1) // P)
            per_b.append((sA, dA, sB, dB))
        blocks.append(per_b)

    cpb = cpba + cpbb
    eidx_all, drow_all = [], []
    for c in range(ncores):
        e = np.zeros((nblk, P, cpba * 8 + cpbb * 8 + cpb), np.int16)
        dr = np.full((nblk, cpb * P), -1.0, np.float16)
        for b in range(nblk):
            sA, dA, sB, dB = blocks[c][b]
            fA = np.zeros(cpba * P, np.int16)
            fA[: len(sA)] = sA
            fB = np.zeros(cpbb * P, np.int16)
            fB[: len(sB)] = sB
            e[b, :, 0 : cpba * 8] = _wrap16(fA)
            e[b, :, cpba * 8 : (cpba + cpbb) * 8] = _wrap16(fB)
            drel = np.full(cpb * P, -1, np.int16)
            drel[: len(dA)] = dA
            drel[cpba * P : cpba * P + len(dB)] = dB
            # column layout [128, cpb]: slot (p, c) = flat[c*128 + p]
            e[b, :, (cpba + cpbb) * 8 :] = drel.reshape(cpb, P).T
            dr[b] = drel.astype(np.float16)
        eidx_all.append(e)
        drow_all.append(dr)
    return eidx_all, drow_all, cpba, cpbb, nblk, npc


def _prepare(h, W, attn_l_w, attn_l_b, attn_r_w, attn_r_b, bias, src, dst):
    h = np.asarray(h, np.float32)
    W = np.ascontiguousarray(np.asarray(W, np.float32))
    attn_l_w = np.asarray(attn_l_w, np.float32)
    attn_l_b = np.asarray(attn_l_b, np.float32)
    attn_r_w = np.asarray(attn_r_w, np.float32)
    attn_r_b = np.asarray(attn_r_b, np.float32)
    bias = np.asarray(bias, np.float32)
    src = np.asarray(src, np.int32)
    dst = np.asarray(dst, np.int32)

    halfr = 25088
    n_pad = 2 * halfr
    eidx_all, drow_all, cpba, cpbb, nblk, npc = preprocess_edges(
        src, dst, N, NCORES, halfr
    )
    nc = build_program(n_pad, halfr, nblk, cpba, cpbb, npc)
    if not nc.is_finalized():
        nc.finalize()

    ht16 = np.zeros((D, n_pad), np.float16)
    ht16[:, :N] = h.T.astype(np.float16)
    arb = np.tile(np.tile(attn_r_w[:, 0], H)[None, :], (P, 1)).astype(np.float16)
    alb = np.tile(np.tile(attn_l_w[:, 0], H)[None, :], (P, 1)).astype(np.float16)
    bsum = np.full((P, 1), attn_r_b[0] + attn_l_b[0], np.float32)
    biasb = np.tile(bias.reshape(1, HF), (P, 1)).astype(np.float32)

    in_maps = []
    for c in range(NCORES):
        in_maps.append(
            {
                "hT16": ht16,
                "W": W,
                "attn_r_bcast": arb,
                "attn_l_bcast": alb,
                "bsum": bsum,
                "bias_bcast": biasb,
                "pidrow": np.array([[c]], np.int32),
                "eidx": eidx_all[c],
                "drow": drow_all[c],
            }
        )
    return nc, in_maps, npc


def _assemble(res, npc):
    outs = [res.results[c]["out"][:npc] for c in range(NCORES)]
    return np.concatenate(outs, axis=0).reshape(N, H, F)


def kernel(h, W, attn_l_w, attn_l_b, attn_r_w, attn_r_b, bias, src, dst):
    nc, in_maps, npc = _prepare(
        h, W, attn_l_w, attn_l_b, attn_r_w, attn_r_b, bias, src, dst
    )
    res = run_bass_kernel_spmd(nc, in_maps, list(range(NCORES)))
    return _assemble(res, npc)


def _ensure_ntff_hook():
    """Dev-harness only: register the axon NTFF profile hook if the container
    image lacks antenv.axon_hooks (trace=True degrades silently otherwise)."""
    import types

    try:
        import antenv.axon_hooks  # noqa: F401

        return
    except ImportError:
        pass
    import antenv

    if "/root/.axon_site" not in sys.path:
        sys.path.insert(0, "/root/.axon_site")
    from trn_agent_boot.trn_boot import _ntff_profile_via_ctypes

    hook = _ntff_profile_via_ctypes("/opt/axon/libaxon_pjrt.so")
    mod = types.ModuleType("antenv.axon_hooks")
    state = {"hook": hook}
    mod.set_axon_ntff_profile_hook = lambda hk: state.__setitem__("hook", hk)
    mod.get_axon_ntff_profile_hook = lambda: state["hook"]
    sys.modules["antenv.axon_hooks"] = mod
    antenv.axon_hooks = mod


def run_timed(inputs, **trace_kwargs):
    """Traced run (dev harness): returns (output, exec_time_ns, results)."""
    import concourse.bass_utils as bu

    _ensure_ntff_hook()
    bu.upload_artifacts = lambda tmpdir: tmpdir
    nc, in_maps, npc = _prepare(**inputs)
    tmpdir = trace_kwargs.pop("tmpdir", "/tmp/gat_trace")
    os.makedirs(tmpdir, exist_ok=True)
    res = run_bass_kernel_spmd(
        nc, in_maps, list(range(NCORES)), trace=True, tmpdir=tmpdir, **trace_kwargs
    )
    return _assemble(res, npc), res.exec_time_ns, res
